# revision 36
# baseline (speedup 1.0000x reference)
"""Causal multi-head attention block (B=16, S=1024, d=1024, H=16) on 8 NeuronCores.

Strategy: data-parallel over batch (2 batches per core), no collectives.
Per-core kernel (fp16 matmuls, fp32 accumulation):
  phase A: transpose x -> xT[d, m] via PE transposes (cast fp32->fp16)
  phase B: QT = Wq @ xT, KT = Wk @ xT (transposed layout [d_out, m]),
           V  = x @ Wv.T (natural layout [m, d_out], packed in 65-wide
           per-head strips with a fused ones column)
  phase C: per (batch, head, q-chunk): scoresT[k, q] = KT.T @ QT on PE,
           exp((s + mask)/8) on ACT, causal mask via 0/1 triangle multiply
           on diagonal blocks + skipping fully-masked blocks, then
           out_unT[dh, q] (+ sum row, from the ones column) = [V|1].T @
           expT accumulated on PE.  Unnormalized outputs + sum rows are
           evacuated on DVE; after every 4 heads ONE batched DVE
           reciprocal_approx_fast computes 1/sums, chunked f16 DMA
           broadcasts fan them out and one in-place DVE multiply per
           (pair, qc) normalizes -- the ACT engine stays exp-only (no
           activation-table swaps).
  phase D: y = attn_outT.T @ WoT (natural layout) -> DRAM; batch-0
           m-tiles are interleaved into attention b1 emission.
Biases: bq/bk are zero by problem spec (ignored); bv/bo folded in exactly
on the host (y += bv @ Wo.T + bo).
"""

import numpy as np

_CACHE: dict = {}

S = 1024
D = 1024
H = 16
DH = 64
BPC = 2           # batches per core
M = BPC * S       # tokens per core
NCORES = 8


def _build_nc():
    import concourse.bass as bass  # noqa: F401
    import concourse.mybir as mybir
    import concourse.tile as tile
    from concourse import bacc
    from concourse.masks import make_identity
    from contextlib import ExitStack

    f32 = mybir.dt.float32
    f16 = mybir.dt.float16
    EXPF = mybir.ActivationFunctionType.Exp

    nc = bacc.Bacc("TRN2", target_bir_lowering=False, debug=False,
                   num_devices=NCORES)

    x_d = nc.dram_tensor("x", [M, D], f32, kind="ExternalInput")
    wq_d = nc.dram_tensor("Wq", [D, D], f32, kind="ExternalInput")
    wk_d = nc.dram_tensor("Wk", [D, D], f32, kind="ExternalInput")
    wv_d = nc.dram_tensor("Wv", [D, D], f32, kind="ExternalInput")
    wo_d = nc.dram_tensor("Wo", [D, D], f32, kind="ExternalInput")
    tri_d = nc.dram_tensor("tri01", [128, 128], f16, kind="ExternalInput")
    y_d = nc.dram_tensor("y", [M, D], f32, kind="ExternalOutput")

    NMT = M // 128        # 16 m-tiles
    NDT = D // 128        # 8 d-tiles
    NMC = M // 512        # 4 m-chunks
    NOC = D // 512        # 2 o-chunks

    with tile.TileContext(nc) as tc, ExitStack() as top:
        consts = top.enter_context(tc.tile_pool(name="consts", bufs=1))
        persist = top.enter_context(tc.tile_pool(name="persist", bufs=1))
        wrot = top.enter_context(tc.tile_pool(name="wrot", bufs=1))
        stage = top.enter_context(tc.tile_pool(name="stage", bufs=2))
        ystage = top.enter_context(tc.tile_pool(name="ystage", bufs=4))
        expp = top.enter_context(tc.tile_pool(name="expp", bufs=12))
        tmpp = top.enter_context(tc.tile_pool(name="tmpp", bufs=3))
        srp = top.enter_context(tc.tile_pool(name="srp", bufs=2))
        bcp = top.enter_context(tc.tile_pool(name="bcp", bufs=3))
        grpp = top.enter_context(tc.tile_pool(name="grpp", bufs=3))
        # PSUM: 3 + 3 + 2 = 8 banks
        psAcc = top.enter_context(tc.tile_pool(name="psAcc", bufs=3, space="PSUM"))
        psS = top.enter_context(tc.tile_pool(name="psS", bufs=3, space="PSUM"))
        psO = top.enter_context(tc.tile_pool(name="psO", bufs=2, space="PSUM"))

        ident = consts.tile([128, 128], f32, tag="ident")
        make_identity(nc, ident)
        tri01 = consts.tile([128, 128], f16, tag="tri")
        nc.sync.dma_start(out=tri01, in_=tri_d[:, :])

        # persistent activations (fp16)
        QT = persist.tile([128, NDT, M], f16, tag="QT")    # [o, m] transposed
        KT = persist.tile([128, NDT, M], f16, tag="KT")
        V = persist.tile([128, NMT, H * 65], f16, tag="V")  # [m, head strips]

        def load_transposed(dst, dram, ncols):
            """dst[:, i_tile, c*128:(c+1)*128] = dram[c*128:(c+1)*128, :].T
            dst: [128, NDT, ncols] fp16; dram: [ncols, D] fp32."""
            for rt in range(ncols // 128):
                st = stage.tile([128, D], f32, tag="stage")
                nc.sync.dma_start(out=st, in_=dram[rt * 128:(rt + 1) * 128, :])
                for g in range(NDT // 4):
                    pt = psAcc.tile([128, 512], f32, tag="psAcc")
                    for c in range(4):
                        ib = g * 4 + c
                        nc.tensor.transpose(
                            pt[:, c * 128:(c + 1) * 128],
                            st[:, ib * 128:(ib + 1) * 128], ident)
                    nc.scalar.copy(
                        out=dst[:, g * 4:g * 4 + 4, rt * 128:(rt + 1) * 128],
                        in_=pt.rearrange("p (a b) -> p a b", b=128))

        # ---------- phases A+B: projections ----------
        # xT and AO share one slot: xT's last reader is the V projection,
        # AO's first writer is the attention evacuation (WAR via slot reuse)
        xTp = top.enter_context(tc.tile_pool(name="xTp", bufs=1))
        xT = xTp.tile([128, NDT, M], f16, tag="xTAO")
        load_transposed(xT, x_d, M)

        # Q and K projections -> transposed layout (batch-0 m-chunks first)
        for w_dram, dst in ((wq_d, QT), (wk_d, KT)):
            WT = wrot.tile([128, NDT, D], f16, tag="WT")
            load_transposed(WT, w_dram, D)
            for mc in range(NMC):
                for ot in range(NDT):
                    pp = psAcc.tile([128, 512], f32, tag="psAcc")
                    for it in range(NDT):
                        nc.tensor.matmul(
                            pp,
                            WT[:, it, ot * 128:(ot + 1) * 128],
                            xT[:, it, mc * 512:(mc + 1) * 512],
                            start=(it == 0), stop=(it == NDT - 1))
                    nc.scalar.copy(
                        out=dst[:, ot, mc * 512:(mc + 1) * 512], in_=pp)

        # V projection -> natural layout in 65-wide head strips:
        # every head h: [V(64) | ones] at cols h*65..h*65+64
        WT = wrot.tile([128, NDT, D], f16, tag="WT")
        load_transposed(WT, wv_d, D)
        for oc in range(NOC):        # oc=0 (heads 0-7) first: attention b0
            for mt in range(NMT):    # pair 0 only needs the oc=0 strips
                v2 = V[:, mt, :].rearrange("p (a c) -> p a c", c=65)
                if oc == 0:
                    nc.gpsimd.memset(v2[:, :, 64], 1.0)
                pp = psAcc.tile([128, 512], f32, tag="psAcc")
                for it in range(NDT):
                    nc.tensor.matmul(
                        pp,
                        xT[:, it, mt * 128:(mt + 1) * 128],
                        WT[:, it, oc * 512:(oc + 1) * 512],
                        start=(it == 0), stop=(it == NDT - 1))
                nc.scalar.copy(
                    out=v2[:, 8 * oc:8 * oc + 8, 0:64],
                    in_=pp.rearrange("p (a c) -> p a c", c=64))

        # attn out (transposed layout), reuses xT's SBUF slot
        AO = xTp.tile([128, NDT, M], f16, tag="xTAO")

        # ---------- phase C: attention (per batch) ----------
        def normalize_group(b, hgrp, sgrp):
            """Reciprocal + normalize for heads 4*hgrp..4*hgrp+3 of batch b
            (their 8 sum rows, collected in sgrp, are complete).
            sgrp row local = (h%4)*2 + qc."""
            rg32 = grpp.tile([8, 512], f32, tag="rg32")
            rg = grpp.tile([8, 512], f16, tag="rg")
            nc.vector.reciprocal_approx_fast(out=rg32, in_=sgrp)
            nc.vector.tensor_copy(out=rg, in_=rg32)
            for lp in range(2):
                p = 2 * hgrp + lp                # head pair == dt block
                for qc in range(2):
                    loc_e = (2 * lp) * 2 + qc    # local rows in the group
                    loc_o = loc_e + 2
                    q0 = b * S + qc * 512
                    bc = bcp.tile([128, 512], f16, tag="bc")
                    for loc, p0 in ((loc_e, 0), (loc_o, 64)):
                        r1 = rg[loc:loc + 1, :]
                        for ch in range(4):  # 4 parallel column chunks
                            rc = r1[:, ch * 128:(ch + 1) * 128]
                            rsrc = bass.AP(
                                tensor=rc.tensor, offset=rc.offset,
                                ap=[list(rc.ap[0]), [0, 64]]
                                + [list(a) for a in rc.ap[1:]])
                            nc.sync.dma_start(
                                out=bc[p0:p0 + 64, ch * 128:(ch + 1) * 128],
                                in_=rsrc)
                    nc.vector.tensor_mul(
                        out=AO[:, p, q0:q0 + 512],
                        in0=AO[:, p, q0:q0 + 512], in1=bc)

        def attention_batch(b, interleave=None):
            for h in range(H):
                thq = h // 2
                po = (h % 2) * 64     # partition offset of this head
                even = (h % 2 == 0)
                if h % 4 == 0:
                    sgrp = grpp.tile([8, 512], f32, name="sgrp", tag="sgrp")
                for qc in range(2):
                    loc = (h % 4) * 2 + qc    # row within the recip group
                    q0 = b * S + qc * 512     # global q start (m coords)
                    ps_o = psO.tile([128, 512], f32, tag="psO")
                    nkt = (qc + 1) * 4
                    # pass 1: all score blocks + exp (PE & ACT pipelined)
                    exts = []
                    for kt in range(nkt):
                        k0 = kt * 128
                        off = max(0, k0 - qc * 512)
                        kg = b * S + k0
                        ps_s = psS.tile([128, 512], f32, tag="psS")
                        nc.tensor.matmul(
                            ps_s[:, off:512],
                            KT[po:po + 64, thq, kg:kg + 128],
                            QT[po:po + 64, thq, q0 + off:q0 + 512],
                            start=True, stop=True)
                        ex = expp.tile([128, 512], f16, tag="exp")
                        nc.scalar.activation(
                            out=ex[:, off:512], in_=ps_s[:, off:512],
                            func=EXPF, scale=0.125)
                        if k0 >= qc * 512:  # diagonal block: 0/1 triangle
                            nc.vector.tensor_mul(
                                ex[:, off:off + 128],
                                ex[:, off:off + 128], tri01)
                        exts.append((ex, off, kt))
                    # pass 2: AV accumulation (+ sums via ones column)
                    for ex, off, kt in exts:
                        mtv = b * (S // 128) + kt
                        nc.tensor.matmul(
                            ps_o[0:65, off:512],
                            V[:, mtv, h * 65:h * 65 + 65],
                            ex[:, off:512],
                            start=(kt == 0), stop=(kt == nkt - 1))
                    # evacuate unnormalized output + sum row (DVE)
                    srow = srp.tile([65, 512], f32, tag="srow")
                    nc.vector.tensor_copy(out=srow[64:65, :],
                                          in_=ps_o[64:65, :])
                    nc.sync.dma_start(out=sgrp[loc:loc + 1, :],
                                      in_=srow[64:65, :])
                    if even:
                        nc.vector.tensor_copy(
                            out=AO[0:64, thq, q0:q0 + 512],
                            in_=ps_o[0:64, :])
                    else:
                        tmp = tmpp.tile([64, 512], f16, tag="tmp")
                        nc.vector.tensor_copy(out=tmp, in_=ps_o[0:64, :])
                        nc.sync.dma_start(
                            out=AO[64:128, thq, q0:q0 + 512], in_=tmp)
                if h % 4 == 3:
                    normalize_group(b, h // 4, sgrp)
                if interleave is not None:
                    interleave(h)

        def out_proj_tile(mt, WoT):
            for oc in range(NOC):
                pp = psAcc.tile([128, 512], f32, tag="psAcc")
                for dt_ in range(NDT):
                    nc.tensor.matmul(
                        pp,
                        AO[:, dt_, mt * 128:(mt + 1) * 128],
                        WoT[:, dt_, oc * 512:(oc + 1) * 512],
                        start=(dt_ == 0), stop=(dt_ == NDT - 1))
                ys = ystage.tile([128, 512], f32, tag="ys")
                nc.scalar.copy(out=ys, in_=pp)
                nc.sync.dma_start(
                    out=y_d[mt * 128:(mt + 1) * 128,
                            oc * 512:(oc + 1) * 512], in_=ys)

        def out_proj_batch(b, WoT):
            for mt in range(b * (NMT // 2), (b + 1) * (NMT // 2)):
                out_proj_tile(mt, WoT)

        attention_batch(0)
        WoT = wrot.tile([128, NDT, D], f16, tag="WT")
        load_transposed(WoT, wo_d, D)

        # attention b1 with phase-D m-tiles of b0 interleaved (they hide in
        # the ACT-bound stretches of attention)
        def _ilv(h):
            if h % 2 == 1:
                out_proj_tile(h // 2, WoT)
        attention_batch(1, interleave=_ilv)
        out_proj_batch(1, WoT)

    nc.compile()
    return nc


def _tri01():
    # tri01[dk, dq] = 1 where k <= q (allowed), else 0
    return np.triu(np.ones((128, 128), np.float16))


def _get_nc():
    if "nc" not in _CACHE:
        _CACHE["nc"] = _build_nc()
    return _CACHE["nc"]


def kernel(x, Wq, bq, Wk, bk, Wv, bv, Wo, bo):
    from concourse.bass_utils import run_bass_kernel_spmd

    x = np.ascontiguousarray(np.asarray(x, dtype=np.float32))
    B = x.shape[0]
    assert x.shape == (B, S, D) and B == NCORES * BPC
    Wq = np.ascontiguousarray(np.asarray(Wq, dtype=np.float32))
    Wk = np.ascontiguousarray(np.asarray(Wk, dtype=np.float32))
    Wv = np.ascontiguousarray(np.asarray(Wv, dtype=np.float32))
    Wo = np.ascontiguousarray(np.asarray(Wo, dtype=np.float32))

    nc = _get_nc()
    shards = x.reshape(NCORES, M, D)
    tri = _tri01()
    in_maps = [
        {"x": shards[c], "Wq": Wq, "Wk": Wk, "Wv": Wv, "Wo": Wo, "tri01": tri}
        for c in range(NCORES)
    ]
    res = run_bass_kernel_spmd(nc, in_maps, core_ids=list(range(NCORES)))
    y = np.stack([res.results[c]["y"] for c in range(NCORES)])
    y = y.reshape(B, S, D)

    # exact host-side fold of bv and bo (bq/bk are zero by problem spec)
    bias = (np.asarray(bv, np.float32) @ np.asarray(Wo, np.float32).T
            + np.asarray(bo, np.float32))
    if np.any(bias):
        y = y + bias
    return y.astype(np.float32)


# revision 37
# speedup vs baseline: 1.0300x; 1.0300x over previous
"""Causal multi-head attention block (B=16, S=1024, d=1024, H=16) on 8 NeuronCores.

Strategy: data-parallel over batch (2 batches per core), no collectives.
Per-core kernel (fp16 matmuls, fp32 accumulation):
  phase A: transpose x -> xT[d, m] via PE transposes (cast fp32->fp16)
  phase B: QT = Wq @ xT, KT = Wk @ xT (transposed layout [d_out, m]),
           V  = x @ Wv.T (natural layout [m, d_out], packed in 65-wide
           per-head strips with a fused ones column)
  phase C: per (batch, head, q-chunk): scoresT[k, q] = KT.T @ QT on PE,
           exp((s + mask)/8) on ACT, causal mask via 0/1 triangle multiply
           on diagonal blocks + skipping fully-masked blocks, then
           out_unT[dh, q] (+ sum row, from the ones column) = [V|1].T @
           expT accumulated on PE.  Unnormalized outputs + sum rows are
           evacuated on DVE; after every 4 heads ONE batched DVE
           reciprocal_approx_fast computes 1/sums, chunked f16 DMA
           broadcasts fan them out and one in-place DVE multiply per
           (pair, qc) normalizes -- the ACT engine stays exp-only (no
           activation-table swaps).
  phase D: y = attn_outT.T @ WoT (natural layout) -> DRAM; batch-0
           m-tiles are interleaved into attention b1 emission.
Biases: bq/bk are zero by problem spec (ignored); bv/bo folded in exactly
on the host (y += bv @ Wo.T + bo).
"""

import numpy as np

_CACHE: dict = {}

S = 1024
D = 1024
H = 16
DH = 64
BPC = 2           # batches per core
M = BPC * S       # tokens per core
NCORES = 8


def _build_nc():
    import concourse.bass as bass  # noqa: F401
    import concourse.mybir as mybir
    import concourse.tile as tile
    from concourse import bacc
    from concourse.masks import make_identity
    from contextlib import ExitStack

    f32 = mybir.dt.float32
    f16 = mybir.dt.float16
    EXPF = mybir.ActivationFunctionType.Exp

    nc = bacc.Bacc("TRN2", target_bir_lowering=False, debug=False,
                   num_devices=NCORES)

    x_d = nc.dram_tensor("x", [M, D], f32, kind="ExternalInput")
    wq_d = nc.dram_tensor("Wq", [D, D], f32, kind="ExternalInput")
    wk_d = nc.dram_tensor("Wk", [D, D], f32, kind="ExternalInput")
    wv_d = nc.dram_tensor("Wv", [D, D], f32, kind="ExternalInput")
    wo_d = nc.dram_tensor("Wo", [D, D], f32, kind="ExternalInput")
    tri_d = nc.dram_tensor("tri01", [128, 128], f16, kind="ExternalInput")
    y_d = nc.dram_tensor("y", [M, D], f32, kind="ExternalOutput")

    NMT = M // 128        # 16 m-tiles
    NDT = D // 128        # 8 d-tiles
    NMC = M // 512        # 4 m-chunks
    NOC = D // 512        # 2 o-chunks

    with tile.TileContext(nc) as tc, ExitStack() as top:
        consts = top.enter_context(tc.tile_pool(name="consts", bufs=1))
        persist = top.enter_context(tc.tile_pool(name="persist", bufs=1))
        wrot = top.enter_context(tc.tile_pool(name="wrot", bufs=1))
        stage = top.enter_context(tc.tile_pool(name="stage", bufs=2))
        ystage = top.enter_context(tc.tile_pool(name="ystage", bufs=2))
        expp = top.enter_context(tc.tile_pool(name="expp", bufs=16))
        tmpp = top.enter_context(tc.tile_pool(name="tmpp", bufs=3))
        srp = top.enter_context(tc.tile_pool(name="srp", bufs=2))
        bcp = top.enter_context(tc.tile_pool(name="bcp", bufs=3))
        grpp = top.enter_context(tc.tile_pool(name="grpp", bufs=3))
        # PSUM: 2 + 4 + 2 = 8 banks
        psAcc = top.enter_context(tc.tile_pool(name="psAcc", bufs=2, space="PSUM"))
        psS = top.enter_context(tc.tile_pool(name="psS", bufs=4, space="PSUM"))
        psO = top.enter_context(tc.tile_pool(name="psO", bufs=2, space="PSUM"))

        ident = consts.tile([128, 128], f32, tag="ident")
        make_identity(nc, ident)
        tri01 = consts.tile([128, 128], f16, tag="tri")
        nc.sync.dma_start(out=tri01, in_=tri_d[:, :])

        # persistent activations (fp16)
        QT = persist.tile([128, NDT, M], f16, tag="QT")    # [o, m] transposed
        KT = persist.tile([128, NDT, M], f16, tag="KT")
        V = persist.tile([128, NMT, H * 65], f16, tag="V")  # [m, head strips]

        def load_transposed(dst, dram, ncols):
            """dst[:, i_tile, c*128:(c+1)*128] = dram[c*128:(c+1)*128, :].T
            dst: [128, NDT, ncols] fp16; dram: [ncols, D] fp32."""
            for rt in range(ncols // 128):
                st = stage.tile([128, D], f32, tag="stage")
                nc.sync.dma_start(out=st, in_=dram[rt * 128:(rt + 1) * 128, :])
                for g in range(NDT // 4):
                    pt = psAcc.tile([128, 512], f32, tag="psAcc")
                    for c in range(4):
                        ib = g * 4 + c
                        nc.tensor.transpose(
                            pt[:, c * 128:(c + 1) * 128],
                            st[:, ib * 128:(ib + 1) * 128], ident)
                    nc.scalar.copy(
                        out=dst[:, g * 4:g * 4 + 4, rt * 128:(rt + 1) * 128],
                        in_=pt.rearrange("p (a b) -> p a b", b=128))

        # ---------- phases A+B: projections ----------
        # xT and AO share one slot: xT's last reader is the V projection,
        # AO's first writer is the attention evacuation (WAR via slot reuse)
        xTp = top.enter_context(tc.tile_pool(name="xTp", bufs=1))
        xT = xTp.tile([128, NDT, M], f16, tag="xTAO")
        load_transposed(xT, x_d, M)

        # Q and K projections -> transposed layout (batch-0 m-chunks first)
        for w_dram, dst in ((wq_d, QT), (wk_d, KT)):
            WT = wrot.tile([128, NDT, D], f16, tag="WT")
            load_transposed(WT, w_dram, D)
            for mc in range(NMC):
                for ot in range(NDT):
                    pp = psAcc.tile([128, 512], f32, tag="psAcc")
                    for it in range(NDT):
                        nc.tensor.matmul(
                            pp,
                            WT[:, it, ot * 128:(ot + 1) * 128],
                            xT[:, it, mc * 512:(mc + 1) * 512],
                            start=(it == 0), stop=(it == NDT - 1))
                    nc.scalar.copy(
                        out=dst[:, ot, mc * 512:(mc + 1) * 512], in_=pp)

        # V projection -> natural layout in 65-wide head strips:
        # every head h: [V(64) | ones] at cols h*65..h*65+64
        WT = wrot.tile([128, NDT, D], f16, tag="WT")
        load_transposed(WT, wv_d, D)
        for mt in range(NMT):
            v2 = V[:, mt, :].rearrange("p (a c) -> p a c", c=65)
            nc.gpsimd.memset(v2[:, :, 64], 1.0)
            for oc in range(NOC):
                pp = psAcc.tile([128, 512], f32, tag="psAcc")
                for it in range(NDT):
                    nc.tensor.matmul(
                        pp,
                        xT[:, it, mt * 128:(mt + 1) * 128],
                        WT[:, it, oc * 512:(oc + 1) * 512],
                        start=(it == 0), stop=(it == NDT - 1))
                nc.scalar.copy(
                    out=v2[:, 8 * oc:8 * oc + 8, 0:64],
                    in_=pp.rearrange("p (a c) -> p a c", c=64))

        # attn out (transposed layout), reuses xT's SBUF slot
        AO = xTp.tile([128, NDT, M], f16, tag="xTAO")

        # ---------- phase C: attention (per batch) ----------
        def normalize_group(b, hgrp, sgrp):
            """Reciprocal + normalize for heads 4*hgrp..4*hgrp+3 of batch b
            (their 8 sum rows, collected in sgrp, are complete).
            sgrp row local = (h%4)*2 + qc."""
            rg32 = grpp.tile([8, 512], f32, tag="rg32")
            rg = grpp.tile([8, 512], f16, tag="rg")
            nc.vector.reciprocal_approx_fast(out=rg32, in_=sgrp)
            nc.vector.tensor_copy(out=rg, in_=rg32)
            for lp in range(2):
                p = 2 * hgrp + lp                # head pair == dt block
                for qc in range(2):
                    loc_e = (2 * lp) * 2 + qc    # local rows in the group
                    loc_o = loc_e + 2
                    q0 = b * S + qc * 512
                    bc = bcp.tile([128, 512], f16, tag="bc")
                    for loc, p0 in ((loc_e, 0), (loc_o, 64)):
                        r1 = rg[loc:loc + 1, :]
                        for ch in range(4):  # 4 parallel column chunks
                            rc = r1[:, ch * 128:(ch + 1) * 128]
                            rsrc = bass.AP(
                                tensor=rc.tensor, offset=rc.offset,
                                ap=[list(rc.ap[0]), [0, 64]]
                                + [list(a) for a in rc.ap[1:]])
                            nc.sync.dma_start(
                                out=bc[p0:p0 + 64, ch * 128:(ch + 1) * 128],
                                in_=rsrc)
                    nc.vector.tensor_mul(
                        out=AO[:, p, q0:q0 + 512],
                        in0=AO[:, p, q0:q0 + 512], in1=bc)

        def attention_batch(b, interleave=None):
            for h in range(H):
                thq = h // 2
                po = (h % 2) * 64     # partition offset of this head
                even = (h % 2 == 0)
                if h % 4 == 0:
                    sgrp = grpp.tile([8, 512], f32, name="sgrp", tag="sgrp")
                for qc in range(2):
                    loc = (h % 4) * 2 + qc    # row within the recip group
                    q0 = b * S + qc * 512     # global q start (m coords)
                    ps_o = psO.tile([128, 512], f32, tag="psO")
                    nkt = (qc + 1) * 4
                    # pass 1: all score blocks + exp (PE & ACT pipelined)
                    exts = []
                    for kt in range(nkt):
                        k0 = kt * 128
                        off = max(0, k0 - qc * 512)
                        kg = b * S + k0
                        ps_s = psS.tile([128, 512], f32, tag="psS")
                        nc.tensor.matmul(
                            ps_s[:, off:512],
                            KT[po:po + 64, thq, kg:kg + 128],
                            QT[po:po + 64, thq, q0 + off:q0 + 512],
                            start=True, stop=True)
                        ex = expp.tile([128, 512], f16, tag="exp")
                        nc.scalar.activation(
                            out=ex[:, off:512], in_=ps_s[:, off:512],
                            func=EXPF, scale=0.125)
                        if k0 >= qc * 512:  # diagonal block: 0/1 triangle
                            nc.vector.tensor_mul(
                                ex[:, off:off + 128],
                                ex[:, off:off + 128], tri01)
                        exts.append((ex, off, kt))
                    # pass 2: AV accumulation (+ sums via ones column)
                    for ex, off, kt in exts:
                        mtv = b * (S // 128) + kt
                        nc.tensor.matmul(
                            ps_o[0:65, off:512],
                            V[:, mtv, h * 65:h * 65 + 65],
                            ex[:, off:512],
                            start=(kt == 0), stop=(kt == nkt - 1))
                    # evacuate unnormalized output + sum row (DVE)
                    srow = srp.tile([65, 512], f32, tag="srow")
                    nc.vector.tensor_copy(out=srow[64:65, :],
                                          in_=ps_o[64:65, :])
                    nc.sync.dma_start(out=sgrp[loc:loc + 1, :],
                                      in_=srow[64:65, :])
                    if even:
                        nc.vector.tensor_copy(
                            out=AO[0:64, thq, q0:q0 + 512],
                            in_=ps_o[0:64, :])
                    else:
                        tmp = tmpp.tile([64, 512], f16, tag="tmp")
                        nc.vector.tensor_copy(out=tmp, in_=ps_o[0:64, :])
                        nc.sync.dma_start(
                            out=AO[64:128, thq, q0:q0 + 512], in_=tmp)
                if h % 4 == 3:
                    normalize_group(b, h // 4, sgrp)
                if interleave is not None:
                    interleave(h)

        def out_proj_tile(mt, WoT):
            ys = ystage.tile([128, D], f32, tag="ys")
            for oc in range(NOC):
                pp = psAcc.tile([128, 512], f32, tag="psAcc")
                for dt_ in range(NDT):
                    nc.tensor.matmul(
                        pp,
                        AO[:, dt_, mt * 128:(mt + 1) * 128],
                        WoT[:, dt_, oc * 512:(oc + 1) * 512],
                        start=(dt_ == 0), stop=(dt_ == NDT - 1))
                nc.scalar.copy(out=ys[:, oc * 512:(oc + 1) * 512], in_=pp)
            nc.sync.dma_start(out=y_d[mt * 128:(mt + 1) * 128, :], in_=ys)

        def out_proj_batch(b, WoT):
            for mt in range(b * (NMT // 2), (b + 1) * (NMT // 2)):
                out_proj_tile(mt, WoT)

        attention_batch(0)
        WoT = wrot.tile([128, NDT, D], f16, tag="WT")
        load_transposed(WoT, wo_d, D)

        # attention b1 with phase-D m-tiles of b0 interleaved (they hide in
        # the ACT-bound stretches of attention)
        def _ilv(h):
            if h % 2 == 1:
                out_proj_tile(h // 2, WoT)
        attention_batch(1, interleave=_ilv)
        out_proj_batch(1, WoT)

    nc.compile()
    return nc


def _tri01():
    # tri01[dk, dq] = 1 where k <= q (allowed), else 0
    return np.triu(np.ones((128, 128), np.float16))


def _get_nc():
    if "nc" not in _CACHE:
        _CACHE["nc"] = _build_nc()
    return _CACHE["nc"]


def kernel(x, Wq, bq, Wk, bk, Wv, bv, Wo, bo):
    from concourse.bass_utils import run_bass_kernel_spmd

    x = np.ascontiguousarray(np.asarray(x, dtype=np.float32))
    B = x.shape[0]
    assert x.shape == (B, S, D) and B == NCORES * BPC
    Wq = np.ascontiguousarray(np.asarray(Wq, dtype=np.float32))
    Wk = np.ascontiguousarray(np.asarray(Wk, dtype=np.float32))
    Wv = np.ascontiguousarray(np.asarray(Wv, dtype=np.float32))
    Wo = np.ascontiguousarray(np.asarray(Wo, dtype=np.float32))

    nc = _get_nc()
    shards = x.reshape(NCORES, M, D)
    tri = _tri01()
    in_maps = [
        {"x": shards[c], "Wq": Wq, "Wk": Wk, "Wv": Wv, "Wo": Wo, "tri01": tri}
        for c in range(NCORES)
    ]
    res = run_bass_kernel_spmd(nc, in_maps, core_ids=list(range(NCORES)))
    y = np.stack([res.results[c]["y"] for c in range(NCORES)])
    y = y.reshape(B, S, D)

    # exact host-side fold of bv and bo (bq/bk are zero by problem spec)
    bias = (np.asarray(bv, np.float32) @ np.asarray(Wo, np.float32).T
            + np.asarray(bo, np.float32))
    if np.any(bias):
        y = y + bias
    return y.astype(np.float32)
